# revision 19
# baseline (speedup 1.0000x reference)
"""Trainium2 Bass kernel for nn_CrossAttention (16-head cross attention).

Reference computation (fp32, s1=s2=2048, d1=d2=1024, H=16, DK=DV=64):
    q = x1 @ Wq.T ; k = x2 @ Wk.T ; v = x2 @ Wv.T      (per-head reshape)
    attn = softmax(q k^T / 8) per head
    out = LeakyReLU_0.01((attn v) @ Wo.T + bo)

Distribution (8 NeuronCores, tensor-parallel over heads):
  - Each core owns 2 heads: column-shards of Wq/Wk/Wv (128 rows each).
  - Inputs are fed pre-transposed from the host (x1.T, x2.T, W.T) so the
    contraction axis lands on SBUF partitions without any on-device
    transposition of the big activations.
  - Per-head attention in "transposed" orientation: S^T[j,i] tiles ->
    exp on ACT (no max subtraction: |scores|<~3 by construction) ->
    O'^T = [V|1]^T @ P^T fusing the softmax denominator into the matmul.
    Scores are K=64 matmuls sliced at the head's partition range (64h)
    straight out of the merged qT/kT projection buffers -- no zero pads.
  - The schedule is ACT(exp)-paced: scores for tile t+1 are emitted
    before PV of tile t so the exp stream never starves; the gp1
    projections trickle into the attention tensor-slack one matmul at a
    time via generators.
  - Normalized heads (bf16) are exchanged with AllToAlls (0.5 MB/core)
    so each core ends up with ALL heads for its 256-row slice of s1;
    the output projection uses the full Wo (no reduction needed). The
    out-proj lhsT lives in one [128, ...] tile (even heads rows 0:64,
    odd rows 64:128) so each exchange's half runs as unpadded K=64
    matmuls.
  - Epilogue: bias via K=1 ones-row matmul, LeakyReLU on ACT (exact,
    alpha immediate); output is the core's 256-row slice; the host
    concatenates the 8 slices.
"""

import os as _os
import numpy as np

import concourse.bass as bass
import concourse.mybir as mybir
import concourse.tile as tile
from concourse import bacc
from concourse import bass_utils
from concourse.masks import make_identity

NC_CORES = 8
S1 = 2048
S2 = 2048
D1 = 1024
D2 = 1024
H, DK, DV = 16, 64, 64
HPC = H // NC_CORES          # heads per core = 2
EPC = HPC * DK               # projection dims per core = 128
SPC = S1 // NC_CORES         # output rows per core = 256
P = 128
F32 = mybir.dt.float32
F32R = mybir.dt.float32r
BF16 = mybir.dt.bfloat16
ACT_EXP = mybir.ActivationFunctionType.Exp
ACT_LRELU = mybir.ActivationFunctionType.Lrelu
MAX = mybir.AluOpType.max

NEG_SLOPE = 0.01
SCALE = 1.0 / np.sqrt(np.float32(DK))   # 0.125

S2_T = S2 // P               # 16 key tiles
KD1 = D1 // P                # 8 contraction tiles for projections
KDV = (H * DV) // P          # 8 contraction tiles for out projection

MM_DTYPE = "bf16"

QUARTERS = [(0, 0), (0, 1), (1, 0), (1, 1)]


def build(mm_dtype: str = MM_DTYPE, single_core: bool = False):
    """single_core=True swaps the AllToAll for a local DMA copy (its exact
    1-core semantics) so the kernel can run in TimelineSim."""
    mmdt = {"bf16": BF16, "f32r": F32R, "f32": F32}[mm_dtype]
    nc = bacc.Bacc("TRN2", target_bir_lowering=False, debug=False,
                   num_devices=1 if single_core else NC_CORES)

    x1T = nc.dram_tensor("x1T", [D1, S1], mmdt, kind="ExternalInput")
    x2T = nc.dram_tensor("x2T", [D2, S2], mmdt, kind="ExternalInput")
    wqT = nc.dram_tensor("wqT", [D1, EPC], mmdt, kind="ExternalInput")
    wkT = nc.dram_tensor("wkT", [D2, EPC], mmdt, kind="ExternalInput")
    wvT = nc.dram_tensor("wvT", [D2, EPC], mmdt, kind="ExternalInput")
    woT = nc.dram_tensor("woT", [H * DV, D1], mmdt, kind="ExternalInput")
    bo_bc = nc.dram_tensor("bo_bc", [1, D1], F32, kind="ExternalInput")
    out = nc.dram_tensor("out", [SPC, D1], F32, kind="ExternalOutput")
    a2a_dt = BF16 if mmdt == BF16 else F32
    a2a_in = [nc.dram_tensor(f"a2a_in{h}", [NC_CORES * DV, SPC], a2a_dt,
                             kind="Internal") for h in range(HPC)]
    a2a_out = [nc.dram_tensor(f"a2a_out{h}", [NC_CORES * DV, SPC], a2a_dt,
                              kind="Internal") for h in range(HPC)]

    _ptb = int(_os.environ.get("PTB", "6"))
    _xtb = int(_os.environ.get("XTB", "26"))
    _psb = int(_os.environ.get("PSB", "2"))
    _pob = int(_os.environ.get("POB", "2"))
    RFAST = _os.environ.get("RFAST", "1") == "1"
    LRELU = _os.environ.get("LRELU", "1") == "1"
    DUMP = _os.environ.get("DUMP", "0") == "1"
    dmp = {}
    if DUMP:
        for nm, shp in (("qTd", [P, S1]), ("kTd", [P, S2]),
                        ("vTd", [P, S2]), ("vPd", [P, S2_T * 2 * (DV + 1)]),
                        ("oT0d", [DV, S1]), ("oT1d", [DV, S1]),
                        ("ao0d", [NC_CORES * DV, SPC]),
                        ("ao1d", [NC_CORES * DV, SPC]),
                        ("lt0d", [P, KDV * P]), ("lt1d", [P, KDV * P])):
            dmp[nm] = nc.dram_tensor(nm, shp, a2a_dt, kind="ExternalOutput")

    with tile.TileContext(nc) as tc:
        with (
            tc.tile_pool(name="const", bufs=1) as cpool,
            tc.tile_pool(name="res", bufs=1) as rpool,
            tc.tile_pool(name="xin", bufs=_xtb) as xpool,
            tc.tile_pool(name="lhs", bufs=2) as lpool,
            tc.tile_pool(name="pt", bufs=_ptb) as ptpool,
            tc.tile_pool(name="ytmp", bufs=2) as ypool,
            tc.tile_pool(name="norm", bufs=2) as npool,
            tc.tile_pool(name="ps", bufs=_psb, space="PSUM") as pspool,
            tc.tile_pool(name="po", bufs=_pob, space="PSUM") as popool,
            tc.tile_pool(name="pj", bufs=2, space="PSUM") as pjpool,
        ):
            # ---------------- constants (wo/bias deferred) ----------------
            ident = cpool.tile([P, P], mmdt)
            make_identity(nc, ident[:])
            wq_sb = cpool.tile([P, KD1, EPC], mmdt)
            wk_sb = cpool.tile([P, KD1, EPC], mmdt)
            wv_sb = cpool.tile([P, KD1, EPC], mmdt)
            nc.sync.dma_start(wk_sb[:], wkT.rearrange("(o p) m -> p o m", p=P))
            nc.sync.dma_start(wv_sb[:], wvT.rearrange("(o p) m -> p o m", p=P))
            nc.sync.dma_start(wq_sb[:], wqT.rearrange("(o p) m -> p o m", p=P))
            wo_sb = cpool.tile([P, KDV, D1], mmdt)
            bo_r = cpool.tile([1, D1], F32R)
            ones_r = cpool.tile([1, P], F32R)
            nc.vector.memset(ones_r[:].bitcast(F32), 1.0)
            # all-ones row: K=1 matmul lhsT broadcasting the softmax
            # denominator across the DV output partitions
            ones_t = cpool.tile([1, DV], F32R)
            nc.vector.memset(ones_t[:].bitcast(F32), 1.0)

            # ---------------- residents ----------------
            vT = rpool.tile([P, S2], mmdt, name="vT")
            # merged per-head-pair q^T/k^T: head h at rows 64h:64h+64.
            # Scores are K=64 matmuls at base partition 64h (auto
            # tile_position) -- the unused half is never read, no pads.
            qT = rpool.tile([P, S1], mmdt, name="qT")
            kT = rpool.tile([P, S2], mmdt, name="kT")
            # V natural + ones column, per key tile: [j, (v_h0|1|v_h1|1)]
            vP = rpool.tile([P, S2_T, 2 * (DV + 1)], mmdt)
            oTh = [rpool.tile([DV, S1], a2a_dt, name=f"oT{h}")
                   for h in range(HPC)]
            nc.vector.memset(vP[:, :, DV:DV + 1], 1.0)
            nc.vector.memset(vP[:, :, 2 * DV + 1:2 * DV + 2], 1.0)

            # ---------------- projections ----------------
            # K and V share one pass over x2T (each x2 tile DMA'd once).
            x2v = x2T.rearrange("(o p) i -> p o i", p=P)
            x1v = x1T.rearrange("(o p) i -> p o i", p=P)

            def x_load(gp):
                # all 16 tiles of this gp issued up front; the large xpool
                # keeps every buffer free so transfers run back-to-back
                for dg in range(KD1):
                    xt = xpool.tile([P, 1024], mmdt, tag="xt",
                                    name=f"xt2_{gp}_{dg}")
                    eng = (nc.gpsimd, nc.scalar, nc.sync)[dg % 3] \
                        if gp == 0 else nc.gpsimd
                    eng.dma_start(xt[:], x2v[:, dg, gp * 1024:(gp + 1) * 1024])
                    x2_tiles[(gp, dg)] = xt
                for dg in range(KD1):
                    xt = xpool.tile([P, 1024], mmdt, tag="xt",
                                    name=f"xt1_{gp}_{dg}")
                    nc.sync.dma_start(
                        xt[:], x1v[:, dg, gp * 1024:(gp + 1) * 1024])
                    x1_tiles[(gp, dg)] = xt

            x2_tiles = {}
            x1_tiles = {}

            def kv_proj_mms(gp):
                """Generator: yields after each matmul pair so the emission
                loop can trickle gp1 into attention tensor-slack. gp0 runs
                before attention (pspool free); gp1 shares popool's slots
                (only 2 are free mid-quarter, and fillers are never pulled
                while two quarters' po accumulators overlap)."""
                pool, tg = pjpool, "pj"
                for sg in range(2):
                    ssl = slice(sg * 512, (sg + 1) * 512)
                    pk = pool.tile([P, 512], F32, tag=tg,
                                   name=f"pk{gp}_{sg}")
                    pv = pool.tile([P, 512], F32, tag=tg,
                                   name=f"pv{gp}_{sg}")
                    for d in range(KD1):
                        xt = x2_tiles[(gp, d)]
                        nc.tensor.matmul(pk[:], wk_sb[:, d, :], xt[:, ssl],
                                         start=(d == 0), stop=(d == KD1 - 1))
                        nc.tensor.matmul(pv[:], wv_sb[:, d, :], xt[:, ssl],
                                         start=(d == 0), stop=(d == KD1 - 1))
                        yield
                    g0 = gp * 1024 + sg * 512
                    nc.vector.tensor_copy(kT[:, g0:g0 + 512], pk[:])
                    nc.vector.tensor_copy(vT[:, g0:g0 + 512], pv[:])
                    yield

            def q_proj_mms(gp):
                for sg in range(2):
                    ssl = slice(sg * 512, (sg + 1) * 512)
                    pq = pjpool.tile([P, 512], F32, tag="pj",
                                     name=f"pq{gp}_{sg}")
                    for d in range(KD1):
                        xt = x1_tiles[(gp, d)]
                        nc.tensor.matmul(pq[:], wq_sb[:, d, :], xt[:, ssl],
                                         start=(d == 0), stop=(d == KD1 - 1))
                        yield
                    g0 = gp * 1024 + sg * 512
                    nc.vector.tensor_copy(qT[:, g0:g0 + 512], pq[:])
                    yield

            def v_transpose_mms(half):
                for kk in range(2):
                    ptr = pjpool.tile([P, 512], mmdt, tag="pj",
                                      name=f"ptr{half}_{kk}")
                    for k in range(4):
                        t = 8 * half + 4 * kk + k
                        nc.tensor.transpose(
                            ptr[:, k * P:(k + 1) * P],
                            vT[:, t * P:(t + 1) * P], ident[:])
                    yield
                    for k in range(4):
                        t = 8 * half + 4 * kk + k
                        nc.vector.tensor_copy(
                            vP[:, t, 0:DV], ptr[:, k * P:k * P + DV])
                        nc.vector.tensor_copy(
                            vP[:, t, DV + 1:2 * DV + 1],
                            ptr[:, k * P + DV:(k + 1) * P])
                    yield

            # ---------------- attention steps ----------------
            po_tiles = {}
            ptt_tiles = {}

            def score_step(h, ih, t):
                dat = slice(h * DK, (h + 1) * DK)
                sps = pspool.tile([P, 1024], F32, tag="ps",
                                  name=f"sps_{h}_{ih}_{t}")
                for sg in range(2):
                    i0 = ih * 1024 + sg * 512
                    nc.tensor.matmul(
                        sps[:, sg * 512:(sg + 1) * 512],
                        kT[dat, t * P:(t + 1) * P],
                        qT[dat, i0:i0 + 512],
                        start=True, stop=True)
                ptt = ptpool.tile([P, 1024], mmdt, tag="ptt",
                                  name=f"ptt_{h}_{ih}_{t}")
                nc.scalar.activation(ptt[:], sps[:], ACT_EXP,
                                     scale=float(SCALE))
                ptt_tiles[(h, ih, t)] = ptt

            def pv_step(h, ih, t):
                if (h, ih) not in po_tiles:
                    po_tiles[(h, ih)] = [
                        popool.tile([DV + 1, 512], F32, tag="po",
                                    name=f"po_{h}_{ih}_{gg}")
                        for gg in range(2)]
                po = po_tiles[(h, ih)]
                ptt = ptt_tiles.pop((h, ih, t))
                for sg in range(2):
                    nc.tensor.matmul(
                        po[sg][:],
                        vP[:, t, h * (DV + 1):(h + 1) * (DV + 1)],
                        ptt[:, sg * 512:(sg + 1) * 512],
                        start=(t == 0), stop=(t == S2_T - 1))

            sr_tiles = {}

            def fin_recip(h, ih):
                # stage the denominator row (f32r round only on the fast
                # path; the reciprocal happens post-broadcast where the
                # custom-DVE op is correct -- it misreads PSUM sources at
                # partition base 64)
                po = po_tiles[(h, ih)]
                srs = []
                for gg in range(2):
                    g = ih * 2 + gg
                    sr = npool.tile([1, 512], F32R, tag="sr",
                                    name=f"sr_{h}_{g}")
                    if RFAST:
                        nc.vector.tensor_copy(sr[:], po[gg][DV:DV + 1, :])
                    else:
                        sf = npool.tile([1, 512], F32, tag="sf",
                                        name=f"sf_{h}_{g}")
                        nc.vector.reciprocal(sf[:], po[gg][DV:DV + 1, :])
                        nc.vector.tensor_copy(sr[:], sf[:])
                    srs.append(sr)
                sr_tiles[(h, ih)] = srs

            def fin_apply(h, ih):
                # broadcast Z over DV partitions (K=1 matmul), then 1/Z on
                # 64 partitions and one fused multiply out of po PSUM into
                # the bf16 exchange buffer
                po = po_tiles.pop((h, ih))
                srs = sr_tiles.pop((h, ih))
                for gg in range(2):
                    g = ih * 2 + gg
                    gs = slice(g * 512, (g + 1) * 512)
                    bc = pjpool.tile([DV, 512], F32, tag="pj",
                                     name=f"bc_{h}_{g}")
                    nc.tensor.matmul(bc[:], ones_t[:], srs[gg][:],
                                     start=True, stop=True)
                    if RFAST:
                        rf = npool.tile([DV, 512], F32, tag="rf",
                                        name=f"rf_{h}_{g}")
                        nc.vector.reciprocal_approx_fast(rf[:], bc[:])
                        nc.vector.tensor_mul(
                            oTh[h][:, gs], po[gg][0:DV, :], rf[:])
                    else:
                        nc.vector.tensor_copy(oTh[h][:, gs], po[gg][0:DV, :])
                        nc.vector.tensor_mul(
                            oTh[h][:, gs], oTh[h][:, gs], bc[:])

            def scatter_half(h, half):
                jsl = slice(half * 4, (half + 1) * 4)
                nc.sync.dma_start(
                    a2a_in[h].rearrange("(j p) i -> p j i", p=DV)[:, jsl, :],
                    oTh[h][:].rearrange("p (j i) -> p j i", j=NC_CORES)
                             [:, jsl, :])

            def exchange(h):
                if single_core:
                    nc.sync.dma_start(a2a_out[h][:], a2a_in[h][:])
                else:
                    nc.gpsimd.collective_compute(
                        "AllToAll", mybir.AluOpType.bypass,
                        replica_groups=[list(range(NC_CORES))],
                        ins=[a2a_in[h][:].opt()],
                        outs=[a2a_out[h][:].opt()],
                    )

            # out-proj lhsT: one [128, k, 128] tile per row block; even
            # heads (a2a 0) land at rows 0:64, odd heads at 64:128. Each
            # phase is an unpadded K=64 matmul at that partition base.
            ltAB = [lpool.tile([P, KDV, P], mmdt, tag="lt", name=f"lt{it}")
                    for it in range(SPC // P)]

            def lt_load(h):
                hs = slice(h * DV, (h + 1) * DV)
                for it in range(SPC // P):
                    # must stay on gpsimd: the collective's a2a_out write is
                    # ordered against readers only within its own queue
                    nc.gpsimd.dma_start(
                        ltAB[it][hs, :, :],
                        a2a_out[h].rearrange("(k p) i -> p k i", p=DV)
                                  [:, :, it * P:(it + 1) * P])

            pys = []

            def outproj_phase(h):
                # phase h: contributions of heads with parity h (K=64 rows
                # at base partition 64h). Runs right after its exchange.
                hs = slice(h * DV, (h + 1) * DV)
                for it in range(SPC // P):
                    if h == 0:
                        pys.append(pspool.tile([P, D1], F32, tag="ps",
                                               name=f"py{it}"))
                    py = pys[it]
                    for k in range(KDV):
                        for ng in range(2):
                            nc.tensor.matmul(
                                py[:, ng * 512:(ng + 1) * 512],
                                ltAB[it][hs, k, :],
                                wo_sb[hs, k, ng * 512:(ng + 1) * 512],
                                start=(h == 0 and k == 0), stop=False,
                                skip_group_check=True)
                    if h == 1:
                        for ng in range(2):
                            nc.tensor.matmul(
                                py[:, ng * 512:(ng + 1) * 512],
                                ones_r[:],
                                bo_r[:, ng * 512:(ng + 1) * 512],
                                start=False, stop=True,
                                skip_group_check=True)
                        ysb = ypool.tile([P, D1], F32, tag="ysb")
                        if LRELU:
                            nc.scalar.activation(ysb[:], py[:], ACT_LRELU,
                                                 alpha=NEG_SLOPE)
                        else:
                            yml = ypool.tile([P, D1], F32, tag="yml")
                            nc.vector.tensor_scalar_mul(yml[:], py[:],
                                                        NEG_SLOPE)
                            nc.vector.tensor_tensor(ysb[:], py[:], yml[:],
                                                    MAX)
                        nc.sync.dma_start(out[it * P:(it + 1) * P, :],
                                          ysb[:])

            def load_wo():
                nc.sync.dma_start(wo_sb[:],
                                  woT.rearrange("(o p) m -> p o m", p=P))
                nc.gpsimd.dma_start(bo_r[:], bo_bc[0:1, :])

            # ---------------- emission ----------------
            # Startup: gp0 data + projections, serial (nothing else to
            # overlap); gp1 x-loads issue immediately after gp0's.
            x_load(0)
            x_load(1)
            fillers = []  # generator queue drained into attention slack
            for mm in kv_proj_mms(0):
                pass
            for mm in q_proj_mms(0):
                pass
            for mm in v_transpose_mms(0):
                pass
            load_wo()
            # order matters: vt(1) must emit the vP copies for key tiles
            # 8-15 before pv_step(0,0,8..) reads them
            if _os.environ.get("NOFILL", "0") == "1":
                for _gen in (kv_proj_mms(1), v_transpose_mms(1),
                             q_proj_mms(1)):
                    for _ in _gen:
                        pass
            else:
                fillers.extend([kv_proj_mms(1), v_transpose_mms(1),
                                q_proj_mms(1)])

            def pull_filler(n=1):
                for _ in range(n):
                    while fillers:
                        try:
                            next(fillers[0])
                            return
                        except StopIteration:
                            fillers.pop(0)

            # Linear tile walk, scores leading PV by one tile so the exp
            # stream never waits. At a quarter boundary the previous
            # quarter's fin chain (fin_recip DVE round, fin_apply bc
            # matmuls) is emitted BEFORE the new quarter's first pv_step:
            # pv allocates the new po accumulators out of the slots that
            # fin_apply frees, so the bc matmul must sit ahead of it in the
            # FIFO tensor queue (reversing them deadlocks). PV then catches
            # up with a 3-tile burst at t=3.
            for qi, (h, ih) in enumerate(QUARTERS):
                prev = QUARTERS[qi - 1] if qi else None
                for t in range(S2_T):
                    score_step(h, ih, t)
                    if prev is None:
                        if t > 0:
                            pv_step(h, ih, t - 1)
                            pull_filler(3)
                        if t == S2_T - 1:
                            while fillers:
                                pull_filler()
                        continue
                    if t == 0:
                        pv_step(prev[0], prev[1], S2_T - 1)
                    elif t == 1:
                        fin_recip(*prev)
                    elif t == 2:
                        fin_apply(*prev)
                    elif t == 3:
                        for tb in range(3):
                            pv_step(h, ih, tb)
                        scatter_half(*prev)
                        if prev == (0, 1):
                            exchange(0)
                            lt_load(0)
                    else:
                        pv_step(h, ih, t - 1)
                        if t != S2_T - 1:
                            pull_filler()
            pv_step(1, 1, S2_T - 1)
            while fillers:
                pull_filler()
            if DUMP:
                nc.sync.dma_start(dmp["qTd"][:, :], qT[:])
                nc.sync.dma_start(dmp["kTd"][:, :], kT[:])
                nc.sync.dma_start(dmp["vTd"][:, :], vT[:])
                nc.sync.dma_start(dmp["vPd"][:, :],
                                  vP[:].rearrange("p a b -> p (a b)"))
            fin_recip(1, 1)
            fin_apply(1, 1)
            scatter_half(1, 1)
            exchange(1)
            if DUMP:
                nc.sync.dma_start(dmp["oT0d"][:, :], oTh[0][:])
                nc.sync.dma_start(dmp["oT1d"][:, :], oTh[1][:])
            outproj_phase(0)
            lt_load(1)
            outproj_phase(1)
            if DUMP:
                nc.gpsimd.dma_start(dmp["ao0d"][:, :], a2a_out[0][:, :])
                nc.gpsimd.dma_start(dmp["ao1d"][:, :], a2a_out[1][:, :])
                for _it in range(SPC // P):
                    nc.gpsimd.dma_start(
                        dmp[f"lt{_it}d"][:, :],
                        ltAB[_it][:].rearrange("p a b -> p (a b)"))

    nc.compile()
    return nc


_NC_CACHE = {}


def _get_nc():
    if "nc" not in _NC_CACHE:
        _NC_CACHE["nc"] = build()
    return _NC_CACHE["nc"]


def make_in_maps(x1, x2, Wq, Wk, Wv, Wo, bo, mm_dtype: str = MM_DTYPE):
    import ml_dtypes
    cast = (lambda a: a.astype(ml_dtypes.bfloat16)) if mm_dtype == "bf16" \
        else (lambda a: a)
    x1 = np.asarray(x1, dtype=np.float32)
    x2 = np.asarray(x2, dtype=np.float32)
    Wq = np.asarray(Wq, dtype=np.float32)
    Wk = np.asarray(Wk, dtype=np.float32)
    Wv = np.asarray(Wv, dtype=np.float32)
    Wo = np.asarray(Wo, dtype=np.float32)
    bo = np.asarray(bo, dtype=np.float32)
    x1T = cast(np.ascontiguousarray(x1.T))
    x2T = cast(np.ascontiguousarray(x2.T))
    woT = cast(np.ascontiguousarray(Wo.T))
    bo_bc = np.ascontiguousarray(bo.reshape(1, D1))
    in_maps = []
    for c in range(NC_CORES):
        sl = slice(EPC * c, EPC * (c + 1))
        in_maps.append({
            "x1T": x1T,
            "x2T": x2T,
            "wqT": cast(np.ascontiguousarray(Wq[sl, :].T)),
            "wkT": cast(np.ascontiguousarray(Wk[sl, :].T)),
            "wvT": cast(np.ascontiguousarray(Wv[sl, :].T)),
            "woT": woT,
            "bo_bc": bo_bc,
        })
    return in_maps


def _install_profile_shim():
    """The image's antenv lacks axon_hooks; shim it so trace=True can pull
    NTFF profiles (exec_time_ns) through the axon tunnel."""
    import sys as _sys
    import types as _types
    try:
        from antenv.axon_hooks import get_axon_ntff_profile_hook  # noqa: F401
        return
    except ImportError:
        pass
    try:
        from trn_agent_boot.trn_boot import _ntff_profile_via_ctypes
        hook = _ntff_profile_via_ctypes("/opt/axon/libaxon_pjrt.so")
        mod = _types.ModuleType("antenv.axon_hooks")
        mod.get_axon_ntff_profile_hook = lambda: hook
        mod.set_axon_ntff_profile_hook = lambda h: None
        _sys.modules["antenv.axon_hooks"] = mod
        bass_utils.upload_artifacts = lambda tmpdir: tmpdir
    except Exception:
        pass


def run(inputs, trace=False):
    if trace:
        _install_profile_shim()
    nc = _get_nc()
    in_maps = make_in_maps(**inputs)
    res = bass_utils.run_bass_kernel_spmd(
        nc, in_maps, core_ids=list(range(NC_CORES)), trace=trace)
    full = np.concatenate(
        [res.results[c]["out"] for c in range(NC_CORES)], axis=0)
    return full, res


def kernel(**inputs):
    full, _ = run(inputs, trace=False)
    return full


# revision 21
# speedup vs baseline: 1.3718x; 1.3718x over previous
"""Trainium2 Bass kernel for nn_CrossAttention (16-head cross attention).

Reference computation (fp32, s1=s2=2048, d1=d2=1024, H=16, DK=DV=64):
    q = x1 @ Wq.T ; k = x2 @ Wk.T ; v = x2 @ Wv.T      (per-head reshape)
    attn = softmax(q k^T / 8) per head
    out = LeakyReLU_0.01((attn v) @ Wo.T + bo)

Distribution (8 NeuronCores, tensor-parallel over heads):
  - Each core owns 2 heads: column-shards of Wq/Wk/Wv (128 rows each).
  - Inputs are fed pre-transposed from the host (x1.T, x2.T, W.T) so the
    contraction axis lands on SBUF partitions without any on-device
    transposition of the big activations.
  - Per-head attention in "transposed" orientation: S^T[j,i] tiles ->
    exp on ACT (no max subtraction: |scores|<~3 by construction) ->
    O'^T = [V|1]^T @ P^T fusing the softmax denominator into the matmul.
    Scores are K=64 matmuls sliced at the head's partition range (64h)
    straight out of the merged qT/kT projection buffers -- no zero pads.
  - The schedule is ACT(exp)-paced: scores for tile t+1 are emitted
    before PV of tile t so the exp stream never starves; the gp1
    projections trickle into the attention tensor-slack one matmul at a
    time via generators.
  - Normalized heads (bf16) are exchanged with AllToAlls (0.5 MB/core)
    so each core ends up with ALL heads for its 256-row slice of s1;
    the output projection uses the full Wo (no reduction needed). The
    out-proj lhsT lives in one [128, ...] tile (even heads rows 0:64,
    odd rows 64:128) so each exchange's half runs as unpadded K=64
    matmuls.
  - Epilogue: bias via K=1 ones-row matmul, LeakyReLU on ACT (exact,
    alpha immediate); output is the core's 256-row slice; the host
    concatenates the 8 slices.
"""

import os as _os
import numpy as np

import concourse.bass as bass
import concourse.mybir as mybir
import concourse.tile as tile
from concourse import bacc
from concourse import bass_utils
from concourse.masks import make_identity

NC_CORES = 8
S1 = 2048
S2 = 2048
D1 = 1024
D2 = 1024
H, DK, DV = 16, 64, 64
HPC = H // NC_CORES          # heads per core = 2
EPC = HPC * DK               # projection dims per core = 128
SPC = S1 // NC_CORES         # output rows per core = 256
P = 128
F32 = mybir.dt.float32
F32R = mybir.dt.float32r
BF16 = mybir.dt.bfloat16
ACT_EXP = mybir.ActivationFunctionType.Exp
ACT_LRELU = mybir.ActivationFunctionType.Lrelu
MAX = mybir.AluOpType.max

NEG_SLOPE = 0.01
SCALE = 1.0 / np.sqrt(np.float32(DK))   # 0.125

S2_T = S2 // P               # 16 key tiles
KD1 = D1 // P                # 8 contraction tiles for projections
KDV = (H * DV) // P          # 8 contraction tiles for out projection

MM_DTYPE = "bf16"

QUARTERS = [(0, 0), (0, 1), (1, 0), (1, 1)]


def build(mm_dtype: str = MM_DTYPE, single_core: bool = False):
    """single_core=True swaps the AllToAll for a local DMA copy (its exact
    1-core semantics) so the kernel can run in TimelineSim."""
    mmdt = {"bf16": BF16, "f32r": F32R, "f32": F32}[mm_dtype]
    nc = bacc.Bacc("TRN2", target_bir_lowering=False, debug=False,
                   num_devices=1 if single_core else NC_CORES)

    x1T = nc.dram_tensor("x1T", [D1, S1], mmdt, kind="ExternalInput")
    x2T = nc.dram_tensor("x2T", [D2, S2], mmdt, kind="ExternalInput")
    wqT = nc.dram_tensor("wqT", [D1, EPC], mmdt, kind="ExternalInput")
    wkT = nc.dram_tensor("wkT", [D2, EPC], mmdt, kind="ExternalInput")
    wvT = nc.dram_tensor("wvT", [D2, EPC], mmdt, kind="ExternalInput")
    woT = nc.dram_tensor("woT", [H * DV, D1], mmdt, kind="ExternalInput")
    bo_bc = nc.dram_tensor("bo_bc", [1, D1], F32, kind="ExternalInput")
    out = nc.dram_tensor("out", [SPC, D1], F32, kind="ExternalOutput")
    a2a_dt = BF16 if mmdt == BF16 else F32
    a2a_in = [nc.dram_tensor(f"a2a_in{h}", [NC_CORES * DV, SPC], a2a_dt,
                             kind="Internal") for h in range(HPC)]
    a2a_out = [nc.dram_tensor(f"a2a_out{h}", [NC_CORES * DV, SPC], a2a_dt,
                              kind="Internal") for h in range(HPC)]

    _ptb = int(_os.environ.get("PTB", "6"))
    _xtb = int(_os.environ.get("XTB", "32"))
    _psb = int(_os.environ.get("PSB", "2"))
    _pob = int(_os.environ.get("POB", "2"))
    RFAST = _os.environ.get("RFAST", "1") == "1"
    LRELU = _os.environ.get("LRELU", "1") == "1"
    DUMP = _os.environ.get("DUMP", "0") == "1"
    dmp = {}
    if DUMP:
        for nm, shp in (("qTd", [P, S1]), ("kTd", [P, S2]),
                        ("vTd", [P, S2]), ("vPd", [P, S2_T * 2 * (DV + 1)]),
                        ("oT0d", [DV, S1]), ("oT1d", [DV, S1]),
                        ("ao0d", [NC_CORES * DV, SPC]),
                        ("ao1d", [NC_CORES * DV, SPC]),
                        ("lt0d", [P, KDV * P]), ("lt1d", [P, KDV * P])):
            dmp[nm] = nc.dram_tensor(nm, shp, a2a_dt, kind="ExternalOutput")

    with tile.TileContext(nc) as tc:
        with (
            tc.tile_pool(name="const", bufs=1) as cpool,
            tc.tile_pool(name="res", bufs=1) as rpool,
            tc.tile_pool(name="xin", bufs=_xtb) as xpool,
            tc.tile_pool(name="lhs", bufs=2) as lpool,
            tc.tile_pool(name="pt", bufs=_ptb) as ptpool,
            tc.tile_pool(name="ytmp", bufs=2) as ypool,
            tc.tile_pool(name="norm", bufs=2) as npool,
            tc.tile_pool(name="ps", bufs=_psb, space="PSUM") as pspool,
            tc.tile_pool(name="po", bufs=_pob, space="PSUM") as popool,
            tc.tile_pool(name="pj", bufs=2, space="PSUM") as pjpool,
        ):
            # ---------------- constants (wo/bias deferred) ----------------
            ident = cpool.tile([P, P], mmdt)
            make_identity(nc, ident[:])
            wq_sb = cpool.tile([P, KD1, EPC], mmdt)
            wk_sb = cpool.tile([P, KD1, EPC], mmdt)
            wv_sb = cpool.tile([P, KD1, EPC], mmdt)
            nc.sync.dma_start(wk_sb[:], wkT.rearrange("(o p) m -> p o m", p=P))
            nc.sync.dma_start(wv_sb[:], wvT.rearrange("(o p) m -> p o m", p=P))
            nc.sync.dma_start(wq_sb[:], wqT.rearrange("(o p) m -> p o m", p=P))
            wo_sb = cpool.tile([P, KDV, D1], mmdt)
            bo_r = cpool.tile([1, D1], F32R)
            ones_r = cpool.tile([1, P], F32R)
            nc.vector.memset(ones_r[:].bitcast(F32), 1.0)
            # all-ones row: K=1 matmul lhsT broadcasting the softmax
            # denominator across the DV output partitions
            ones_t = cpool.tile([1, DV], F32R)
            nc.vector.memset(ones_t[:].bitcast(F32), 1.0)

            # ---------------- residents ----------------
            vT = rpool.tile([P, S2], mmdt, name="vT")
            # merged per-head-pair q^T/k^T: head h at rows 64h:64h+64.
            # Scores are K=64 matmuls at base partition 64h (auto
            # tile_position) -- the unused half is never read, no pads.
            qT = rpool.tile([P, S1], mmdt, name="qT")
            kT = rpool.tile([P, S2], mmdt, name="kT")
            # V natural + ones column, per key tile: [j, (v_h0|1|v_h1|1)]
            vP = rpool.tile([P, S2_T, 2 * (DV + 1)], mmdt)
            oTh = [rpool.tile([DV, S1], a2a_dt, name=f"oT{h}")
                   for h in range(HPC)]
            nc.vector.memset(vP[:, :, DV:DV + 1], 1.0)
            nc.vector.memset(vP[:, :, 2 * DV + 1:2 * DV + 2], 1.0)

            # ---------------- projections ----------------
            # K and V share one pass over x2T (each x2 tile DMA'd once).
            x2v = x2T.rearrange("(o p) i -> p o i", p=P)
            x1v = x1T.rearrange("(o p) i -> p o i", p=P)

            def x_load(gp):
                # all 16 tiles of this gp issued up front; the large xpool
                # keeps every buffer free so transfers run back-to-back
                for dg in range(KD1):
                    xt = xpool.tile([P, 1024], mmdt, tag="xt",
                                    name=f"xt2_{gp}_{dg}")
                    eng = (nc.gpsimd, nc.scalar, nc.sync)[dg % 3] \
                        if gp == 0 else nc.gpsimd
                    eng.dma_start(xt[:], x2v[:, dg, gp * 1024:(gp + 1) * 1024])
                    x2_tiles[(gp, dg)] = xt
                for dg in range(KD1):
                    xt = xpool.tile([P, 1024], mmdt, tag="xt",
                                    name=f"xt1_{gp}_{dg}")
                    nc.sync.dma_start(
                        xt[:], x1v[:, dg, gp * 1024:(gp + 1) * 1024])
                    x1_tiles[(gp, dg)] = xt

            x2_tiles = {}
            x1_tiles = {}

            def kv_proj_mms(gp):
                """Generator: yields after each matmul pair so the emission
                loop can trickle gp1 into attention tensor-slack. gp0 runs
                before attention (pspool free); gp1 shares popool's slots
                (only 2 are free mid-quarter, and fillers are never pulled
                while two quarters' po accumulators overlap)."""
                pool, tg = pjpool, "pj"
                for sg in range(2):
                    ssl = slice(sg * 512, (sg + 1) * 512)
                    pk = pool.tile([P, 512], F32, tag=tg,
                                   name=f"pk{gp}_{sg}")
                    pv = pool.tile([P, 512], F32, tag=tg,
                                   name=f"pv{gp}_{sg}")
                    for d in range(KD1):
                        xt = x2_tiles[(gp, d)]
                        nc.tensor.matmul(pk[:], wk_sb[:, d, :], xt[:, ssl],
                                         start=(d == 0), stop=(d == KD1 - 1))
                        nc.tensor.matmul(pv[:], wv_sb[:, d, :], xt[:, ssl],
                                         start=(d == 0), stop=(d == KD1 - 1))
                        yield
                    g0 = gp * 1024 + sg * 512
                    nc.vector.tensor_copy(kT[:, g0:g0 + 512], pk[:])
                    nc.vector.tensor_copy(vT[:, g0:g0 + 512], pv[:])
                    yield

            def q_proj_mms(gp):
                for sg in range(2):
                    ssl = slice(sg * 512, (sg + 1) * 512)
                    pq = pjpool.tile([P, 512], F32, tag="pj",
                                     name=f"pq{gp}_{sg}")
                    for d in range(KD1):
                        xt = x1_tiles[(gp, d)]
                        nc.tensor.matmul(pq[:], wq_sb[:, d, :], xt[:, ssl],
                                         start=(d == 0), stop=(d == KD1 - 1))
                        yield
                    g0 = gp * 1024 + sg * 512
                    nc.vector.tensor_copy(qT[:, g0:g0 + 512], pq[:])
                    yield

            def v_transpose_mms(half):
                for kk in range(2):
                    ptr = pjpool.tile([P, 512], mmdt, tag="pj",
                                      name=f"ptr{half}_{kk}")
                    for k in range(4):
                        t = 8 * half + 4 * kk + k
                        nc.tensor.transpose(
                            ptr[:, k * P:(k + 1) * P],
                            vT[:, t * P:(t + 1) * P], ident[:])
                    yield
                    for k in range(4):
                        t = 8 * half + 4 * kk + k
                        nc.vector.tensor_copy(
                            vP[:, t, 0:DV], ptr[:, k * P:k * P + DV])
                        nc.vector.tensor_copy(
                            vP[:, t, DV + 1:2 * DV + 1],
                            ptr[:, k * P + DV:(k + 1) * P])
                    yield

            # ---------------- attention steps ----------------
            po_tiles = {}
            ptt_tiles = {}

            def score_step(h, ih, t):
                dat = slice(h * DK, (h + 1) * DK)
                sps = pspool.tile([P, 1024], F32, tag="ps",
                                  name=f"sps_{h}_{ih}_{t}")
                for sg in range(2):
                    i0 = ih * 1024 + sg * 512
                    nc.tensor.matmul(
                        sps[:, sg * 512:(sg + 1) * 512],
                        kT[dat, t * P:(t + 1) * P],
                        qT[dat, i0:i0 + 512],
                        start=True, stop=True)
                ptt = ptpool.tile([P, 1024], mmdt, tag="ptt",
                                  name=f"ptt_{h}_{ih}_{t}")
                nc.scalar.activation(ptt[:], sps[:], ACT_EXP,
                                     scale=float(SCALE))
                ptt_tiles[(h, ih, t)] = ptt

            def pv_step(h, ih, t):
                if (h, ih) not in po_tiles:
                    po_tiles[(h, ih)] = [
                        popool.tile([DV + 1, 512], F32, tag="po",
                                    name=f"po_{h}_{ih}_{gg}")
                        for gg in range(2)]
                po = po_tiles[(h, ih)]
                ptt = ptt_tiles.pop((h, ih, t))
                for sg in range(2):
                    nc.tensor.matmul(
                        po[sg][:],
                        vP[:, t, h * (DV + 1):(h + 1) * (DV + 1)],
                        ptt[:, sg * 512:(sg + 1) * 512],
                        start=(t == 0), stop=(t == S2_T - 1))

            sr_tiles = {}

            def fin_recip(h, ih):
                # stage the denominator row (f32r round only on the fast
                # path; the reciprocal happens post-broadcast where the
                # custom-DVE op is correct -- it misreads PSUM sources at
                # partition base 64)
                po = po_tiles[(h, ih)]
                srs = []
                for gg in range(2):
                    g = ih * 2 + gg
                    sr = npool.tile([1, 512], F32R, tag="sr",
                                    name=f"sr_{h}_{g}")
                    if RFAST:
                        nc.vector.tensor_copy(sr[:], po[gg][DV:DV + 1, :])
                    else:
                        sf = npool.tile([1, 512], F32, tag="sf",
                                        name=f"sf_{h}_{g}")
                        nc.vector.reciprocal(sf[:], po[gg][DV:DV + 1, :])
                        nc.vector.tensor_copy(sr[:], sf[:])
                    srs.append(sr)
                sr_tiles[(h, ih)] = srs

            def fin_apply(h, ih):
                # broadcast Z over DV partitions (K=1 matmul), then 1/Z on
                # 64 partitions and one fused multiply out of po PSUM into
                # the bf16 exchange buffer
                po = po_tiles.pop((h, ih))
                srs = sr_tiles.pop((h, ih))
                for gg in range(2):
                    g = ih * 2 + gg
                    gs = slice(g * 512, (g + 1) * 512)
                    bc = pjpool.tile([DV, 512], F32, tag="pj",
                                     name=f"bc_{h}_{g}")
                    nc.tensor.matmul(bc[:], ones_t[:], srs[gg][:],
                                     start=True, stop=True)
                    if RFAST:
                        rf = npool.tile([DV, 512], F32, tag="rf",
                                        name=f"rf_{h}_{g}")
                        nc.vector.reciprocal_approx_fast(rf[:], bc[:])
                        nc.vector.tensor_mul(
                            oTh[h][:, gs], po[gg][0:DV, :], rf[:])
                    else:
                        nc.vector.tensor_copy(oTh[h][:, gs], po[gg][0:DV, :])
                        nc.vector.tensor_mul(
                            oTh[h][:, gs], oTh[h][:, gs], bc[:])

            def scatter_half(h, half):
                jsl = slice(half * 4, (half + 1) * 4)
                nc.sync.dma_start(
                    a2a_in[h].rearrange("(j p) i -> p j i", p=DV)[:, jsl, :],
                    oTh[h][:].rearrange("p (j i) -> p j i", j=NC_CORES)
                             [:, jsl, :])

            def exchange(h):
                if single_core:
                    nc.sync.dma_start(a2a_out[h][:], a2a_in[h][:])
                else:
                    nc.gpsimd.collective_compute(
                        "AllToAll", mybir.AluOpType.bypass,
                        replica_groups=[list(range(NC_CORES))],
                        ins=[a2a_in[h][:].opt()],
                        outs=[a2a_out[h][:].opt()],
                    )

            # out-proj lhsT: one [128, k, 128] tile per row block; even
            # heads (a2a 0) land at rows 0:64, odd heads at 64:128. Each
            # phase is an unpadded K=64 matmul at that partition base.
            ltAB = [lpool.tile([P, KDV, P], mmdt, tag="lt", name=f"lt{it}")
                    for it in range(SPC // P)]

            def lt_load(h):
                hs = slice(h * DV, (h + 1) * DV)
                for it in range(SPC // P):
                    # must stay on gpsimd: the collective's a2a_out write is
                    # ordered against readers only within its own queue
                    nc.gpsimd.dma_start(
                        ltAB[it][hs, :, :],
                        a2a_out[h].rearrange("(k p) i -> p k i", p=DV)
                                  [:, :, it * P:(it + 1) * P])

            pys = []

            def outproj_phase(h):
                # phase h: contributions of heads with parity h (K=64 rows
                # at base partition 64h). Runs right after its exchange.
                hs = slice(h * DV, (h + 1) * DV)
                for it in range(SPC // P):
                    if h == 0:
                        pys.append(pspool.tile([P, D1], F32, tag="ps",
                                               name=f"py{it}"))
                    py = pys[it]
                    for k in range(KDV):
                        for ng in range(2):
                            nc.tensor.matmul(
                                py[:, ng * 512:(ng + 1) * 512],
                                ltAB[it][hs, k, :],
                                wo_sb[hs, k, ng * 512:(ng + 1) * 512],
                                start=(h == 0 and k == 0), stop=False,
                                skip_group_check=True)
                    if h == 1:
                        for ng in range(2):
                            nc.tensor.matmul(
                                py[:, ng * 512:(ng + 1) * 512],
                                ones_r[:],
                                bo_r[:, ng * 512:(ng + 1) * 512],
                                start=False, stop=True,
                                skip_group_check=True)
                        ysb = ypool.tile([P, D1], F32, tag="ysb")
                        if LRELU:
                            nc.scalar.activation(ysb[:], py[:], ACT_LRELU,
                                                 alpha=NEG_SLOPE)
                        else:
                            yml = ypool.tile([P, D1], F32, tag="yml")
                            nc.vector.tensor_scalar_mul(yml[:], py[:],
                                                        NEG_SLOPE)
                            nc.vector.tensor_tensor(ysb[:], py[:], yml[:],
                                                    MAX)
                        nc.sync.dma_start(out[it * P:(it + 1) * P, :],
                                          ysb[:])

            def load_wo():
                nc.sync.dma_start(wo_sb[:],
                                  woT.rearrange("(o p) m -> p o m", p=P))
                nc.gpsimd.dma_start(bo_r[:], bo_bc[0:1, :])

            # ---------------- emission ----------------
            # Startup: gp0 data + projections, serial (nothing else to
            # overlap); gp1 x-loads issue immediately after gp0's.
            x_load(0)
            x_load(1)
            fillers = []  # generator queue drained into attention slack
            for mm in kv_proj_mms(0):
                pass
            for mm in q_proj_mms(0):
                pass
            for mm in v_transpose_mms(0):
                pass
            load_wo()
            # order matters: vt(1) must emit the vP copies for key tiles
            # 8-15 before pv_step(0,0,8..) reads them
            if _os.environ.get("NOFILL", "0") == "1":
                for _gen in (kv_proj_mms(1), v_transpose_mms(1),
                             q_proj_mms(1)):
                    for _ in _gen:
                        pass
            else:
                fillers.extend([kv_proj_mms(1), v_transpose_mms(1),
                                q_proj_mms(1)])

            def pull_filler(n=1):
                for _ in range(n):
                    while fillers:
                        try:
                            next(fillers[0])
                            return
                        except StopIteration:
                            fillers.pop(0)

            # Linear tile walk, scores leading PV by one tile so the exp
            # stream never waits. At a quarter boundary the previous
            # quarter's fin chain (fin_recip DVE round, fin_apply bc
            # matmuls) is emitted BEFORE the new quarter's first pv_step:
            # pv allocates the new po accumulators out of the slots that
            # fin_apply frees, so the bc matmul must sit ahead of it in the
            # FIFO tensor queue (reversing them deadlocks). PV then catches
            # up with a 3-tile burst at t=3.
            for qi, (h, ih) in enumerate(QUARTERS):
                prev = QUARTERS[qi - 1] if qi else None
                for t in range(S2_T):
                    score_step(h, ih, t)
                    if prev is None:
                        if t > 0:
                            pv_step(h, ih, t - 1)
                        if t == 7:
                            # gp1 projections as one block (the fine-grained
                            # interleave corrupts head-0 attention on HW);
                            # kv(1) must precede score(0,0,8)
                            while fillers:
                                pull_filler()
                        continue
                    if t == 0:
                        pv_step(prev[0], prev[1], S2_T - 1)
                    elif t == 1:
                        fin_recip(*prev)
                    elif t == 2:
                        fin_apply(*prev)
                    elif t == 3:
                        for tb in range(3):
                            pv_step(h, ih, tb)
                        scatter_half(*prev)
                        if prev == (0, 1):
                            exchange(0)
                            lt_load(0)
                    else:
                        pv_step(h, ih, t - 1)
                        if t != S2_T - 1:
                            pull_filler()
            pv_step(1, 1, S2_T - 1)
            while fillers:
                pull_filler()
            if DUMP:
                nc.sync.dma_start(dmp["qTd"][:, :], qT[:])
                nc.sync.dma_start(dmp["kTd"][:, :], kT[:])
                nc.sync.dma_start(dmp["vTd"][:, :], vT[:])
                nc.sync.dma_start(dmp["vPd"][:, :],
                                  vP[:].rearrange("p a b -> p (a b)"))
            fin_recip(1, 1)
            fin_apply(1, 1)
            scatter_half(1, 1)
            exchange(1)
            if DUMP:
                nc.sync.dma_start(dmp["oT0d"][:, :], oTh[0][:])
                nc.sync.dma_start(dmp["oT1d"][:, :], oTh[1][:])
            outproj_phase(0)
            lt_load(1)
            outproj_phase(1)
            if DUMP:
                nc.gpsimd.dma_start(dmp["ao0d"][:, :], a2a_out[0][:, :])
                nc.gpsimd.dma_start(dmp["ao1d"][:, :], a2a_out[1][:, :])
                for _it in range(SPC // P):
                    nc.gpsimd.dma_start(
                        dmp[f"lt{_it}d"][:, :],
                        ltAB[_it][:].rearrange("p a b -> p (a b)"))

    nc.compile()
    return nc


_NC_CACHE = {}


def _get_nc():
    if "nc" not in _NC_CACHE:
        _NC_CACHE["nc"] = build()
    return _NC_CACHE["nc"]


def make_in_maps(x1, x2, Wq, Wk, Wv, Wo, bo, mm_dtype: str = MM_DTYPE):
    import ml_dtypes
    cast = (lambda a: a.astype(ml_dtypes.bfloat16)) if mm_dtype == "bf16" \
        else (lambda a: a)
    x1 = np.asarray(x1, dtype=np.float32)
    x2 = np.asarray(x2, dtype=np.float32)
    Wq = np.asarray(Wq, dtype=np.float32)
    Wk = np.asarray(Wk, dtype=np.float32)
    Wv = np.asarray(Wv, dtype=np.float32)
    Wo = np.asarray(Wo, dtype=np.float32)
    bo = np.asarray(bo, dtype=np.float32)
    x1T = cast(np.ascontiguousarray(x1.T))
    x2T = cast(np.ascontiguousarray(x2.T))
    woT = cast(np.ascontiguousarray(Wo.T))
    bo_bc = np.ascontiguousarray(bo.reshape(1, D1))
    in_maps = []
    for c in range(NC_CORES):
        sl = slice(EPC * c, EPC * (c + 1))
        in_maps.append({
            "x1T": x1T,
            "x2T": x2T,
            "wqT": cast(np.ascontiguousarray(Wq[sl, :].T)),
            "wkT": cast(np.ascontiguousarray(Wk[sl, :].T)),
            "wvT": cast(np.ascontiguousarray(Wv[sl, :].T)),
            "woT": woT,
            "bo_bc": bo_bc,
        })
    return in_maps


def _install_profile_shim():
    """The image's antenv lacks axon_hooks; shim it so trace=True can pull
    NTFF profiles (exec_time_ns) through the axon tunnel."""
    import sys as _sys
    import types as _types
    try:
        from antenv.axon_hooks import get_axon_ntff_profile_hook  # noqa: F401
        return
    except ImportError:
        pass
    try:
        from trn_agent_boot.trn_boot import _ntff_profile_via_ctypes
        hook = _ntff_profile_via_ctypes("/opt/axon/libaxon_pjrt.so")
        mod = _types.ModuleType("antenv.axon_hooks")
        mod.get_axon_ntff_profile_hook = lambda: hook
        mod.set_axon_ntff_profile_hook = lambda h: None
        _sys.modules["antenv.axon_hooks"] = mod
        bass_utils.upload_artifacts = lambda tmpdir: tmpdir
    except Exception:
        pass


def run(inputs, trace=False):
    if trace:
        _install_profile_shim()
    nc = _get_nc()
    in_maps = make_in_maps(**inputs)
    res = bass_utils.run_bass_kernel_spmd(
        nc, in_maps, core_ids=list(range(NC_CORES)), trace=trace)
    full = np.concatenate(
        [res.results[c]["out"] for c in range(NC_CORES)], axis=0)
    return full, res


def kernel(**inputs):
    full, _ = run(inputs, trace=False)
    return full
